# revision 19
# baseline (speedup 1.0000x reference)
"""AttentionBlock (GroupNorm + single-head 4096x4096 attention + residual) on 8 trn2 cores.

Sharding: core = 2*b + h. Data-parallel over batch (B=4), sequence-parallel over
query rows (2 halves of 2048). Each core receives its batch's x transposed to
[C, N] (bf16) with token columns rotated so the core's own query tokens are
columns 0..2047. K/V are computed for all 4096 tokens on both cores of a pair.

v4: everything heavy runs fp8e4 DoubleRow (2 fp8 weights per PE cell,
contraction 256 per instruction, 0.5 cycles/row):
  - groupnorm is applied to x directly (one fused scale+shift DVE op per
    channel tile) producing fp8 xn in pair layout; weights arrive raw fp8
    from the host, so there is no fold arithmetic at all.
  - QKV projections, scores and attn@V are all DoubleRow fp8. The final
    projection runs bf16 (osb/wp) for accuracy.
  - softmax row-sums accumulate on the PE itself: one tiny [1,512] DoubleRow
    matmul per key-pair against a resident fp8 ones-vector, into a dedicated
    PSUM bank per block (removes the 70us DVE reduction chain).
  - exp carries a -2 shift so fp8 softmax weights stay in (0, ~30], far from
    the fp8e4 +-240 clip; the shift cancels in the normalization.
  - K has no bias (softmax shift invariance leaves only the Q-side bias).
  - PSUM: 4 banks O^T accumulators, 2 scores rotation, 1 row-sum, 1 misc.
  - DMA is batched (x in 8, each weight in 1, smalls in 1, y in 1 per block)
    because each dma_start costs ~650ns of serial Sync-queue time.
"""

import numpy as np
import ml_dtypes
from contextlib import ExitStack

import concourse.bacc as bacc
import concourse.mybir as mybir
import concourse.tile as tile
from concourse.bass_utils import run_bass_kernel_spmd

F32 = mybir.dt.float32
F32R = mybir.dt.float32r
BF16 = mybir.dt.bfloat16
F8 = mybir.dt.float8e4
AF = mybir.ActivationFunctionType
OP = mybir.AluOpType
DRM = mybir.MatmulPerfMode.DoubleRow

B, HH, WW, C = 4, 64, 64, 512
NTOK = HH * WW          # 4096 tokens per batch
NOWN = NTOK // 2        # 2048 own query tokens per core
GROUPS = 32
CG = C // GROUPS        # 16 channels per group
EPS = 1e-5
CT = C // 128           # 4 channel tiles
QTOK = 1024             # token quarter for stage B
NQ = NTOK // QTOK       # 4 quarters
JT = NTOK // 128        # 32 key token tiles
JP = JT // 2            # 16 key token pairs (fp8 DoubleRow granularity)
IB = NOWN // 512        # 4 query i-blocks per core
SCALE = float(C) ** -0.5
ESHIFT = 2.0            # exp(s - ESHIFT): keeps fp8 softmax weights <= ~30

_CACHE = {}


def _build_nc():
    if "nc" in _CACHE:
        return _CACHE["nc"]

    nc = bacc.Bacc(trn_type="TRN2")

    xT = nc.dram_tensor("xT", [C, NTOK], BF16, kind="ExternalInput")
    w_ext = {
        n: nc.dram_tensor(n, [C, C], F8, kind="ExternalInput")
        for n in ("wq", "wk", "wv")
    }
    wp_ext = nc.dram_tensor("wp", [C, C], BF16, kind="ExternalInput")
    # gamma/beta/bq/bp column-tiled + gsel, one packed [128, 24] array
    spack_ext = nc.dram_tensor("spack", [128, 24], F32, kind="ExternalInput")
    gselT_ext = nc.dram_tensor("gselT", [8, 128], F32, kind="ExternalInput")
    bv_ext = nc.dram_tensor("bv", [C], F32, kind="ExternalInput")
    yT_ext = nc.dram_tensor("yT", [C, NOWN], F32, kind="ExternalOutput")

    with ExitStack() as ctx:
        tc = ctx.enter_context(tile.TileContext(nc))

        # ---- persistent pools ------------------------------------------------
        smalls = ctx.enter_context(tc.tile_pool(name="smalls", bufs=1))
        gnp = ctx.enter_context(tc.tile_pool(name="gnp", bufs=2))
        ktp = ctx.enter_context(tc.tile_pool(name="ktp", bufs=1))
        qres = ctx.enter_context(tc.tile_pool(name="qres", bufs=1))
        xres = ctx.enter_context(tc.tile_pool(name="xres", bufs=1))
        xnp = ctx.enter_context(tc.tile_pool(name="xnp", bufs=1))
        vsb = ctx.enter_context(tc.tile_pool(name="vsb", bufs=1))
        wpp = ctx.enter_context(tc.tile_pool(name="wpp", bufs=1))

        psA = ctx.enter_context(tc.tile_pool(name="psA", bufs=3, space="PSUM"))
        psO = ctx.enter_context(tc.tile_pool(name="psO", bufs=4, space="PSUM"))
        # one bank shared between stage A's small matmuls and stage C's
        # per-block softmax row-sum accumulator
        psR = ctx.enter_context(tc.tile_pool(name="psR", bufs=1, space="PSUM"))

        # ---- resident x: batched DMAs (2 per channel tile) -------------------
        xall = [
            xres.tile([128, NTOK], BF16, tag=f"xall{t}", name=f"xall{t}")
            for t in range(CT)
        ]
        nc.sync.dma_start(xall[0][:, 0:512], xT[0:128, 0:512])
        nc.sync.dma_start(xall[0][:, 512:2048], xT[0:128, 512:2048])
        nc.sync.dma_start(xall[0][:, 2048:4096], xT[0:128, 2048:4096])
        for t in range(1, CT):
            for h in range(2):
                nc.sync.dma_start(
                    xall[t][:, h * 2048 : (h + 1) * 2048],
                    xT[t * 128 : (t + 1) * 128, h * 2048 : (h + 1) * 2048],
                )

        # ---- small constants (DMA'd before the weights: the fold needs them
        # at ~25us while the weight transfers are only consumed by stage B) --
        spack = smalls.tile([128, 24], F32, tag="spack")
        nc.sync.dma_start(spack[:], spack_ext[:])
        gselT_sb = smalls.tile([8, 128], F32, tag="gselT")
        nc.sync.dma_start(gselT_sb[:], gselT_ext[:])
        bv_row = smalls.tile([1, C], F32, tag="bv_row")
        nc.sync.dma_start(bv_row[:], bv_ext.rearrange("c -> () c"))

        # ---- weights: one DMA each, [128, CT, C] layout ----------------------
        wqkv_ctx = ExitStack()
        wqkvp = wqkv_ctx.enter_context(tc.tile_pool(name="wqkv", bufs=1))
        w8 = {}
        for n in ("wk", "wq", "wv"):
            w8[n] = wqkvp.tile([128, CT, C], F8, tag=f"w{n}", name=f"w{n}")
            nc.sync.dma_start(
                w8[n][:], w_ext[n].rearrange("(t p) c -> p t c", p=128)
            )
        wpt = wpp.tile([128, CT, C], BF16, tag="wp", name="wpt")
        nc.sync.dma_start(wpt[:], wp_ext.rearrange("(t p) c -> p t c", p=128))

        gamma_t = [spack[:, t : t + 1] for t in range(CT)]
        beta_t = [spack[:, 4 + t : 5 + t] for t in range(CT)]
        bq_t = [spack[:, 8 + t : 9 + t] for t in range(CT)]
        bp_t = [spack[:, 12 + t : 13 + t] for t in range(CT)]
        gsel_sb = spack[:, 16:24]

        ones1_f = smalls.tile([1, 128], F32, tag="ones1_f")
        nc.vector.memset(ones1_f, 1.0)
        eps_row = smalls.tile([8, 1], F32, tag="eps_row")
        nc.vector.memset(eps_row, EPS)
        negshift = smalls.tile([128, 1], F32, tag="negshift")
        nc.vector.memset(negshift, -ESHIFT)
        zero_col = smalls.tile([128, 1], F32, tag="zero_col")
        nc.vector.memset(zero_col, 0.0)
        zero8 = smalls.tile([8, 1], F32, tag="zero8")
        nc.vector.memset(zero8, 0.0)
        # dependency-free dummy ACT: pulls the (single) activation table load
        # into the DMA-wait window instead of the stats critical path
        dmy = smalls.tile([8, 1], F32, tag="dmy")
        nc.scalar.activation(dmy[:], eps_row[:], AF.Exp, bias=zero8[:], scale=1.0)
        ones8 = smalls.tile([128, 2, 16], F8, tag="ones8")
        nc.vector.memset(ones8, 1.0)

        # ---- resident xn/K^T/Q^T/V in fp8 pair layout ------------------------
        xn8 = [
            xnp.tile([128, 2, NTOK], F8, tag=f"xn{cp}", name=f"xn{cp}")
            for cp in range(2)
        ]
        KT8 = [
            [
                ktp.tile([128, 2, QTOK], F8, tag=f"kt{cp}q{qq}", name=f"kt{cp}q{qq}")
                for qq in range(NQ)
            ]
            for cp in range(2)
        ]
        q8 = {}
        for q in range(2):
            for cp in range(2):
                for nch in range(2):
                    q8[q, cp, nch] = qres.tile(
                        [128, 2, 512], F8, tag=f"q{q}{cp}{nch}", name=f"q{q}{cp}{nch}"
                    )
        v8 = [
            vsb.tile([128, 2, C], F8, tag=f"v{jp}", name=f"v{jp}")
            for jp in range(JP)
        ]

        # ---- stage A: groupnorm statistics + fp8 normalized x ----------------
        # stats split across engines: DVE bn_stats covers tokens 0..2047,
        # ACT (Identity/Square with free-dim accumulators) covers 2048..4095;
        # the halves are averaged by folding 1/2 into the 1/CG group scalar.
        with nc.named_scope("stats"):
            stats_t = [
                gnp.tile([128, 5, 6], F32, tag=f"stats{t}", name=f"stats{t}")
                for t in range(CT)
            ]
            sx_a, sq_a = [], []
            for t in range(CT):
                for h in range(5):
                    nc.vector.bn_stats(
                        stats_t[t][:, h, :], xall[t][:, h * 512 : (h + 1) * 512]
                    )
                sx = gnp.tile([128, 1], F32, tag=f"sx{t}", name=f"sx{t}")
                sq = gnp.tile([128, 1], F32, tag=f"sq{t}", name=f"sq{t}")
                sc1 = gnp.tile([128, 1536], F8, tag="sc8", name=f"sc1_{t}")
                nc.scalar.activation(
                    sc1[:], xall[t][:, 2560:4096], AF.Identity,
                    bias=zero_col[:], scale=1.0, accum_out=sx[:],
                )
                sc2 = gnp.tile([128, 1536], F8, tag="sc8", name=f"sc2_{t}")
                nc.scalar.activation(
                    sc2[:], xall[t][:, 2560:4096], AF.Square,
                    bias=zero_col[:], scale=1.0, accum_out=sq[:],
                )
                sx_a.append(sx)
                sq_a.append(sq)

            # packed carries raw sums over all 4096 tokens (2560 DVE + 1536
            # ACT); the /4096 folds into the group-mean scalar below.
            packed = gnp.tile([128, 2 * CT], F32, tag="packed")
            for t in range(CT):
                mv = gnp.tile([128, 2], F32, tag="mv")
                nc.vector.bn_aggr(mv[:], stats_t[t][:])
                t1 = gnp.tile([128, 1], F32, tag="t1")
                nc.vector.tensor_scalar_mul(t1[:], mv[:, 0:1], 2560.0)
                nc.vector.tensor_add(
                    packed[:, 2 * t : 2 * t + 1], t1[:], sx_a[t][:]
                )
                tmp = gnp.tile([128, 1], F32, tag="tmp")
                nc.vector.tensor_mul(tmp[:], mv[:, 0:1], mv[:, 0:1])
                e2d = gnp.tile([128, 1], F32, tag="e2d")
                nc.vector.tensor_add(e2d[:], mv[:, 1:2], tmp[:])
                t2 = gnp.tile([128, 1], F32, tag="t2")
                nc.vector.tensor_scalar_mul(t2[:], e2d[:], 2560.0)
                nc.vector.tensor_add(
                    packed[:, 2 * t + 1 : 2 * t + 2], t2[:], sq_a[t][:]
                )

            g_ps = psR.tile([8, 2 * CT], F32, tag="psr")
            nc.tensor.matmul(g_ps[:], gsel_sb, packed[:], start=True, stop=True)
            stat2 = gnp.tile([8, 2 * CT], F32, tag="stat2")
            nc.vector.tensor_scalar_mul(stat2[:], g_ps[:], 1.0 / (CG * 4096.0))
            s2v = stat2.rearrange("g (t two) -> g t two", two=2)
            mu_v = s2v[:, :, 0]
            e2_v = s2v[:, :, 1]
            musq = gnp.tile([8, CT], F32, tag="musq")
            nc.vector.tensor_mul(musq[:], mu_v, mu_v)
            var = gnp.tile([8, CT], F32, tag="var")
            nc.vector.tensor_sub(var[:], e2_v, musq[:])
            # rstd = exp(-0.5*ln(var+eps)): Ln/Exp share the activation
            # table with Identity/Square/Exp, so no table swaps anywhere
            lnv = gnp.tile([8, CT], F32, tag="lnv")
            nc.scalar.activation(lnv[:], var[:], AF.Ln, bias=eps_row[:], scale=1.0)
            rstd = gnp.tile([8, CT], F32, tag="rstd")
            nc.scalar.activation(rstd[:], lnv[:], AF.Exp, bias=zero8[:], scale=-0.5)

            scale_t, shift_t = [], []
            for t in range(CT):
                cat2 = gnp.tile([8, 2], F32, tag="cat2")
                nc.vector.tensor_copy(cat2[:, 0:1], mu_v[:, t : t + 1])
                nc.vector.tensor_copy(cat2[:, 1:2], rstd[:, t : t + 1])
                bc_ps = psR.tile([128, 2], F32, tag="psr")
                nc.tensor.matmul(bc_ps[:], gselT_sb[:], cat2[:], start=True, stop=True)
                sc = gnp.tile([128, 1], F32, tag=f"scale{t}")
                nc.vector.tensor_mul(sc[:], bc_ps[:, 1:2], gamma_t[t])
                tmp2 = gnp.tile([128, 1], F32, tag="tmp2")
                nc.vector.tensor_mul(tmp2[:], bc_ps[:, 0:1], sc[:])
                sh = gnp.tile([128, 1], F32, tag=f"shift{t}")
                nc.vector.tensor_sub(sh[:], beta_t[t], tmp2[:])
                scale_t.append(sc)
                shift_t.append(sh)

            # xn = s*x + t, quantized to fp8 pair layout; emitted per quarter
            # so stage B can start on quarter 0 early
            for q in range(NQ):
                for ci in range(CT):
                    cp, ko = ci // 2, ci % 2
                    nc.vector.tensor_scalar(
                        out=xn8[cp][:, ko, q * QTOK : (q + 1) * QTOK],
                        in0=xall[ci][:, q * QTOK : (q + 1) * QTOK],
                        scalar1=scale_t[ci][:],
                        scalar2=shift_t[ci][:],
                        op0=OP.mult,
                        op1=OP.add,
                    )

            # broadcast bv across partitions
            bv_ps = psR.tile([128, C], F32, tag="psr")
            nc.tensor.matmul(bv_ps[:], ones1_f[:], bv_row[:], start=True, stop=True)
            bv_bc = smalls.tile([128, C], F32, tag="bv_bc")
            nc.vector.tensor_copy(bv_bc[:], bv_ps[:])

        # ---- stage B: fp8 DoubleRow QKV projections --------------------------
        with nc.named_scope("qkv"):
            def emit_kq(q, name, co):
                cp, ko = co // 2, co % 2
                ps_n = [
                    psO.tile(
                        [128, 512], F32, tag="psO", name=f"psB{q}{name}{co}{i}"
                    )
                    for i in range(2)
                ]
                for ci2 in range(2):
                    for nch in range(2):
                        nc.tensor.matmul(
                            ps_n[nch][:],
                            w8[name][:, ci2 * 2 : ci2 * 2 + 2, co * 128 : (co + 1) * 128],
                            xn8[ci2][:, :, q * QTOK + nch * 512 : q * QTOK + (nch + 1) * 512],
                            start=(ci2 == 0),
                            stop=(ci2 == 1),
                            perf_mode=DRM,
                        )
                for nch in range(2):
                    if name == "wk":
                        dst = KT8[cp][q][
                            :,
                            ko,
                            nch * 512 : (nch + 1) * 512,
                        ]
                        if q == NQ - 1:
                            # last quarter: DVE, so the ACT queue drains
                            # before stage C's first exp
                            nc.vector.tensor_copy(dst, ps_n[nch][:])
                        else:
                            nc.scalar.activation(
                                ps_dst := dst,
                                ps_n[nch][:],
                                AF.Identity,
                                bias=zero_col[:],
                                scale=1.0,
                            )
                    else:
                        nc.scalar.activation(
                            q8[q, cp, nch][:, ko, :],
                            ps_n[nch][:],
                            AF.Identity,
                            bias=bq_t[co],
                            scale=1.0,
                        )

            def emit_v(q, jt):
                j = q * (QTOK // 128) + jt
                ps = psA.tile([128, 512], F32, tag="psA", name=f"psV{q}{jt}")
                for ci2 in range(2):
                    nc.tensor.matmul(
                        ps[:],
                        xn8[ci2][:, :, j * 128 : (j + 1) * 128],
                        w8["wv"][:, ci2 * 2 : ci2 * 2 + 2, :],
                        start=(ci2 == 0),
                        stop=(ci2 == 1),
                        perf_mode=DRM,
                    )
                nc.vector.tensor_add(v8[j // 2][:, j % 2, :], ps[:], bv_bc[:])

            # V emission is interleaved into the co loop so the DVE V-adds
            # overlap the K/Q matmuls (psA only has 2 bufs).
            for q in range(NQ):
                for co in range(CT):
                    emit_kq(q, "wk", co)
                    if q < 2:
                        emit_kq(q, "wq", co)
                    emit_v(q, 2 * co)
                    emit_v(q, 2 * co + 1)

        wqkv_ctx.close()

        # ---- stage C: fp8 DoubleRow attention + bf16 projection --------------
        with (
            tc.tile_pool(name="pt", bufs=6) as ptp,
            tc.tile_pool(name="osb", bufs=4) as osbp,
            tc.tile_pool(name="ysb", bufs=2) as ysbp,
            tc.tile_pool(name="rsb", bufs=2) as rsbp,
            nc.named_scope("attn"),
        ):
            def emit_o(state, jp, pt8_t):
                psO_t = state["psO_t"]
                # row-sum on the PE: [1,512] += ones.T @ pt over the pair
                nc.tensor.matmul(
                    state["psr"][:],
                    ones8[:, :, 0:1],
                    pt8_t[:],
                    start=(jp == 0),
                    stop=(jp == JP - 1),
                    perf_mode=DRM,
                )
                for ct in range(CT):
                    nc.tensor.matmul(
                        psO_t[ct][:],
                        v8[jp][:, :, ct * 128 : (ct + 1) * 128],
                        pt8_t[:],
                        start=(jp == 0),
                        stop=(jp == JP - 1),
                        perf_mode=DRM,
                    )

            def epi_r(state):
                rinv = rsbp.tile([1, 512], F32, tag="rinv")
                nc.vector.reciprocal_approx_fast(rinv[:], state["psr"][:])
                state["rinv"] = rinv

            def epi_osb(state):
                osb = []
                for ct in range(CT):
                    o_t = osbp.tile([128, 512], BF16, tag="osb", name=f"osb{ct}")
                    nc.vector.tensor_copy(o_t[:], state["psO_t"][ct][:])
                    osb.append(o_t)
                state["osb"] = osb

            def epi_rb(state):
                rb_sb = rsbp.tile([128, 512], F32, tag="rb_sb")
                nc.gpsimd.partition_broadcast(rb_sb[:], state["rinv"][:])
                state["rb_sb"] = rb_sb
                state["y_t"] = ysbp.tile(
                    [128, CT, 512], F32, tag="ysb", name=f"y{state['ib']}"
                )

            def epi_proj(state, co):
                ib = state["ib"]
                i0 = ib * 512
                qq, nch = ib // 2, ib % 2
                osb, rb_sb, y_t = state["osb"], state["rb_sb"], state["y_t"]
                psY = psA.tile([128, 512], F32, tag="psA")
                for ci in range(CT):
                    nc.tensor.matmul(
                        psY[:],
                        wpt[:, ci, co * 128 : (co + 1) * 128],
                        osb[ci][:],
                        start=(ci == 0),
                        stop=(ci == CT - 1),
                    )
                xr = xall[co][:, qq * QTOK + nch * 512 : qq * QTOK + (nch + 1) * 512]
                y1_t = ysbp.tile([128, 512], F32, tag="y1sb")
                nc.vector.tensor_mul(y1_t[:], psY[:], rb_sb[:])
                nc.vector.scalar_tensor_tensor(
                    out=y_t[:, co, :],
                    in0=y1_t[:],
                    scalar=bp_t[co],
                    in1=xr,
                    op0=OP.add,
                    op1=OP.add,
                )
                if state.get("final"):
                    # tail latency: fire each co's store as soon as it exists
                    nc.sync.dma_start(
                        yT_ext[co * 128 : (co + 1) * 128, i0 : i0 + 512],
                        y_t[:, co, :],
                    )
                elif co == CT - 1:
                    nc.sync.dma_start(
                        yT_ext.rearrange("(t p) n -> p t n", p=128)[
                            :, :, i0 : i0 + 512
                        ],
                        y_t[:],
                    )

            done_state = None
            for ib in range(IB):
                qq, nch = ib // 2, ib % 2
                qblk8 = [q8[qq, cp, nch] for cp in range(2)]
                state = {
                    "ib": ib,
                    "psO_t": [
                        psO.tile([128, 512], F32, tag="psO", name=f"psO_{ib}_{i}")
                        for i in range(CT)
                    ],
                    "psr": psR.tile([1, 512], F32, tag="psr", name=f"psr{ib}"),
                }

                # 2-deep pair pipeline: O(jp-2) is emitted after S(2*jp), so
                # exp(jp-2) has two full pair-groups of cover. The previous
                # block's epilogue pieces ride in the first j-slots.
                pending = []  # [(jp, pt8_tile)]
                for jp in range(JP):
                    pt8_t = ptp.tile([128, 2, 512], F8, tag="pt", name=f"pt{ib}_{jp}")
                    for sl in range(2):
                        j = 2 * jp + sl
                        psS = psA.tile([128, 512], F32, tag="psA")
                        for cp in range(2):
                            nc.tensor.matmul(
                                psS[:],
                                KT8[cp][j // 8][:, :, (j % 8) * 128 : (j % 8 + 1) * 128],
                                qblk8[cp][:],
                                start=(cp == 0),
                                stop=(cp == 1),
                                perf_mode=DRM,
                            )
                        if sl == 0 and len(pending) >= 2:
                            emit_o(state, *pending.pop(0))
                        if done_state is not None:
                            if j == 0:
                                epi_osb(done_state)
                            elif j == 3:
                                epi_rb(done_state)
                            elif 4 <= j <= 7:
                                epi_proj(done_state, j - 4)
                                if j == 7:
                                    done_state = None
                        nc.scalar.activation(
                            pt8_t[:, sl, :], psS[:], AF.Exp, scale=SCALE,
                            bias=negshift[:],
                        )
                    pending.append((jp, pt8_t))
                for item in pending:
                    emit_o(state, *item)
                epi_r(state)
                done_state = state
            done_state["final"] = True
            epi_osb(done_state)
            epi_rb(done_state)
            for co in range(CT):
                epi_proj(done_state, co)

    nc.compile()
    _CACHE["nc"] = nc
    return nc


def _make_in_maps(x, gamma, beta, wq, bq, wk, bk, wv, bv, wp, bp):
    x = np.asarray(x, dtype=np.float32)
    gsel = np.zeros((128, 8), np.float32)
    for p in range(128):
        gsel[p, p // CG % 8] = 1.0

    spack = np.zeros((128, 24), np.float32)
    for i, vec in enumerate((gamma, beta, bq, bp)):
        spack[:, 4 * i : 4 * i + 4] = np.asarray(vec, np.float32).reshape(CT, 128).T
    spack[:, 16:24] = gsel

    shared = {
        "wq": np.asarray(wq, np.float32).astype(ml_dtypes.float8_e4m3),
        "wk": np.asarray(wk, np.float32).astype(ml_dtypes.float8_e4m3),
        "wv": np.asarray(wv, np.float32).astype(ml_dtypes.float8_e4m3),
        "wp": np.asarray(wp, np.float32).astype(ml_dtypes.bfloat16),
        "bv": np.asarray(bv, np.float32),
        "spack": spack,
        "gselT": np.ascontiguousarray(gsel.T),
    }

    in_maps = []
    for core in range(8):
        b, h = core // 2, core % 2
        xT_b = np.ascontiguousarray(x[b].reshape(NTOK, C).T)  # [C, NTOK]
        if h == 1:
            xT_b = np.ascontiguousarray(
                np.concatenate([xT_b[:, NOWN:], xT_b[:, :NOWN]], axis=1)
            )
        in_maps.append({"xT": xT_b.astype(ml_dtypes.bfloat16), **shared})
    return in_maps


def kernel(x, gamma, beta, wq, bq, wk, bk, wv, bv, wp, bp):
    nc = _build_nc()
    in_maps = _make_in_maps(x, gamma, beta, wq, bq, wk, bk, wv, bv, wp, bp)
    _CACHE["in_maps"] = in_maps

    res = run_bass_kernel_spmd(nc, in_maps, core_ids=list(range(8)))

    y = np.empty((B, NTOK, C), np.float32)
    for core in range(8):
        b, h = core // 2, core % 2
        yT = res.results[core]["yT"]  # [C, NOWN]
        y[b, h * NOWN : (h + 1) * NOWN, :] = yT.T
    return y.reshape(B, HH, WW, C)
